# revision 36
# baseline (speedup 1.0000x reference)
"""Trainium2 Bass kernel for nn_BinaryClassifier (FFT-frame-mean + 3-layer MLP).

Math: the reference computes sigmoid(relu(relu(Re(mean_f FFT(x_f)) @ W1.T +
b1) @ W2.T + b2) @ W3.T + b3). Both the frame-mean and the FFT are linear and
only the real part survives, so
    Re(mean_f FFT(x_f)) = (sum_f x_f) @ (C / 31),  C[n,k] = cos(2*pi*n*k/N)
and layer 1 folds to  relu( (sum_f x_f) @ W1c + b1 )  with W1c = C @ W1.T / 31
precomputed on host in float64. The device work is the 31-frame sum (memory
bound: 32.5 MB/core streamed from HBM at ~358 GB/s -> ~91 us floor) plus a
[128,2048] transpose and the tiny MLP.

Sharding: pure data parallel; 1024 batch rows / 8 cores = 128 rows = exactly
one SBUF partition dim per core. Weights replicated (packed into one small
fp32 "wpk" tensor + one bf16 "w1cb" tensor, one DMA semaphore lane each).

Hardware constraints this build navigates (measured on axon trn2):
- One sync-wait per instruction: walrus rejects any instruction with >=2
  semaphore waits. Bacc.compile()'s generate_event_semaphores splits them,
  and explicit dummy "pre-join" matmuls/activations keep the hot-path
  matmuls at a single data-dependency wait.
- In-DMA accumulate (SWDGE accum_op=add) is correct per-DMA but races when
  one DMA revisits its destination, and runs at ~185 GB/s (RMW doubles SBUF
  port traffic) -> not used for the sum.
- GPSIMD shares an exclusive SBUF port lock with DVE, so GPSIMD elementwise
  adds just serialize against DVE. The parallel pair that works is DVE
  (tensor_add, ~2.3us/frame) + PE (identity-stationary matmul accumulating
  into PSUM, ~3.1us/frame warm fp32); frames split 20:11. Tail scheduling
  matters: DVE's in-place add chain is serial, so the last frames are
  interleaved PE/DVE (PE: 27,29; DVE: 26,28,30) — both chains drain the
  final groups in parallel instead of exposing 5 serial DVE adds.
- fp32 matmuls are two half-rate passes; float32r is single-pass at N>=256
  but slower at N=128, so f32r is used only for the 16 [128,128] transposes
  (~3x faster there) with the sum rounded to f32r by the DVE merge (the BIR
  verifier requires f32r matmul inputs to come from a rounding producer).
- A cold ACT sigmoid table load costs ~1.3us inline; a dummy sigmoid at
  kernel start preloads it during streaming.

- W1c (2 MB of the 2.4 MB constants) is shipped in bf16: saves 1.3 MB of
  stream traffic per core and makes the 32 L1 matmuls single-pass. Costs
  precision: max rel err 1.6e-3 (vs 7e-5 all-fp32) - still far inside any
  scale-relative absmax threshold; sum/DFT accumulation stays fp32.

Measured: 109.4 us/core (plus up to ~+18 us under HBM co-tenant contention —
the same NEFF measures bimodally), max relative error 1.6e-3 vs the fp32 jax
reference. Floor: ~93 us of DMA + ~7 us Bass preamble + ~10 us Tile epilogue.
"""

import os
from contextlib import ExitStack

import numpy as np

import concourse.bacc as bacc
import concourse.bass as bass
import concourse.tile as tile
from concourse import mybir
from concourse.bass_utils import run_bass_kernel_spmd

FRAMES = 31
FFT_LEN = 2048
B = 1024
NCORES = 8
BS = B // NCORES  # 128
H1 = 256
H2 = 256
P = 128
KCH = FFT_LEN // P  # 16 contraction chunks for layer 1

F32 = mybir.dt.float32
F32R = mybir.dt.float32r

# column layout of the packed fp32 constants tensor wpk [128, NW]
IDENT0 = 0
W2T0 = IDENT0 + P  # 2 chunks x 256
W3T0 = W2T0 + 2 * H2  # 2 cols
B10 = W3T0 + 2  # 2 cols
B20 = B10 + 2  # 2 cols
B30 = B20 + 2  # 1 col
NW = B30 + 1
NW1B = KCH * H1  # bf16 W1c tensor cols (16 chunks x 256)

# Frame-sum strategies (BASS_SUM_MODE): "dvpe" (default, fastest) = frames
# split DVE:PE; "dve" = all adds on DVE; "pe" = all frames summed on PE via
# identity matmuls; "split" = DVE+GPSIMD (port-locked, slow); "dma" =
# in-flight SWDGE accumulation (races within one DMA: WRONG results, kept
# only as a reference of the experiment).
SUM_MODE = os.environ.get("BASS_SUM_MODE", "dvpe")
# number of feature chunks for the frame-sum stage (dma mode)
NCHUNK = int(os.environ.get("BASS_SUM_CHUNKS", "2"))


def build_nc(sum_mode: str = SUM_MODE) -> bass.Bass:
    # Bacc (not raw Bass): its compile() runs generate_event_semaphores,
    # which splits multi-sem waits to satisfy the 1-wait-per-instruction
    # hardware constraint walrus enforces.
    nc = bacc.Bacc("TRN2", debug=False)

    x_h = nc.dram_tensor("x", [BS, FRAMES * FFT_LEN], F32, kind="ExternalInput")
    wpk_h = nc.dram_tensor("wpk", [P, NW], F32, kind="ExternalInput")
    w1cb_h = nc.dram_tensor(
        "w1cb", [P, NW1B], mybir.dt.bfloat16, kind="ExternalInput"
    )
    out_h = nc.dram_tensor("out", [1, BS], F32, kind="ExternalOutput")

    x = x_h.ap()
    x3 = x.rearrange("p (f n) -> p f n", f=FRAMES)  # [128, 31, 2048]

    with ExitStack() as ctx:
        tc = ctx.enter_context(tile.TileContext(nc))
        singles = ctx.enter_context(tc.tile_pool(name="singles", bufs=1))
        state = ctx.enter_context(tc.tile_pool(name="state", bufs=1))
        pwork = ctx.enter_context(tc.tile_pool(name="pwork", bufs=3, space="PSUM"))
        pout = ctx.enter_context(tc.tile_pool(name="pout", bufs=1, space="PSUM"))

        wpk = singles.tile([P, NW], F32)
        nc.sync.dma_start(out=wpk, in_=wpk_h.ap())
        ident = wpk[:, IDENT0 : IDENT0 + P]
        w1cb = singles.tile([P, NW1B], mybir.dt.bfloat16)
        nc.scalar.dma_start(out=w1cb, in_=w1cb_h.ap())

        def w1c(k, m):
            c0 = k * H1 + m * P
            return w1cb[:, c0 : c0 + P]

        def w2t(k, m):
            c0 = W2T0 + k * H2 + m * P
            return wpk[:, c0 : c0 + P]

        # pre-join: make PE and ACT observe the wpk DMA once, so real
        # matmuls/activations only ever wait on their single data dependency
        # (cayman Matmult has one hardware wait slot).
        dummy_ps = pwork.tile([1, 1], F32, tag="pw")
        nc.tensor.matmul(
            dummy_ps,
            lhsT=wpk[:, 0:1],
            rhs=wpk[:, 0:1],
            start=True,
            stop=True,
        )
        dummy_ps1 = pwork.tile([1, 1], F32, tag="pw")
        nc.tensor.matmul(
            dummy_ps1,
            lhsT=w1cb[:, 0:1],
            rhs=w1cb[:, 0:1],
            start=True,
            stop=True,
        )
        scr = state.tile([1, 1], F32, tag="scr")
        nc.scalar.activation(
            scr,
            wpk[0:1, 0:1],
            mybir.ActivationFunctionType.Copy,
            bias=0.0,
            scale=1.0,
        )
        # f32r copy of the identity for single-pass f32r transposes (DVE
        # cast-copy is a legal "rounded to f32r" producer; 0/1 are exact)
        ident_r = singles.tile([P, P], F32R)
        nc.vector.tensor_copy(ident_r, ident)
        # preload the sigmoid activation table during streaming (a cold
        # ACT_TABLE_LOAD costs ~1.3us inline right before the output)
        nc.scalar.activation(
            scr,
            wpk[0:1, 0:1],
            mybir.ActivationFunctionType.Sigmoid,
            bias=0.0,
            scale=1.0,
        )

        # ---- frame sum: s[p, n] = sum_f x[p, f*2048 + n] ----
        # (the +1 scratch column only matters for the legacy "dma" mode: a
        # dummy PE matmul reads it after the memset so PE observes the DVE
        # tick without a WAR hazard against the accumulate DMAs)
        s_dtype = F32R if sum_mode == "dvpe" else F32
        s_sb_pad = state.tile([P, FFT_LEN + 1], s_dtype, tag="s_sb")
        s_sb = s_sb_pad[:, 0:FFT_LEN]

        if sum_mode == "dma":
            nc.vector.memset(s_sb_pad, 0.0)
            dummy_ps2 = pwork.tile([1, 1], F32, tag="pw")
            nc.tensor.matmul(
                dummy_ps2,
                lhsT=s_sb_pad[:, FFT_LEN : FFT_LEN + 1],
                rhs=s_sb_pad[:, FFT_LEN : FFT_LEN + 1],
                start=True,
                stop=True,
            )
            W = FFT_LEN // NCHUNK
            for c in range(NCHUNK):
                cs = slice(c * W, (c + 1) * W)
                base = s_sb[:, cs]
                # destination AP revisits the same [128, W] range FRAMES times
                # (zero-stride middle dim); accum_op=add turns that into a sum.
                dst = bass.AP(
                    tensor=base.tensor,
                    offset=base.offset,
                    ap=[list(base.ap[0]), [0, FRAMES], list(base.ap[1])],
                )
                nc.gpsimd.dma_start(
                    out=dst, in_=x3[:, :, cs], accum_op=mybir.AluOpType.add
                )
        elif sum_mode == "dve":
            # plain HWDGE loads of 2-frame groups; all 31 adds on DVE
            # (engine-side SBUF ports — no DMA port contention; ~2.1us per
            # [128,2048] fp32 add -> ~65us, under the ~91us DMA floor).
            G = 2
            NG = (FRAMES + G - 1) // G
            frames_pool = ctx.enter_context(tc.tile_pool(name="frames", bufs=4))
            first = True
            for g in range(NG):
                f0 = g * G
                nf = min(G, FRAMES - f0)
                xg = frames_pool.tile([P, G * FFT_LEN], F32, tag="xg")
                nc.sync.dma_start(
                    out=xg[:, : nf * FFT_LEN], in_=x3[:, f0 : f0 + nf, :]
                )
                for j in range(nf):
                    sl = xg[:, j * FFT_LEN : (j + 1) * FFT_LEN]
                    if first:
                        nc.vector.tensor_copy(s_sb, sl)
                        first = False
                    else:
                        nc.vector.tensor_add(s_sb, s_sb, sl)
        elif sum_mode == "split":
            # plain HWDGE loads of 2-frame groups, alternating between the two
            # HW-DGE rings (SP + ACT) for issue parallelism; the 31 elementwise
            # adds are split DVE:GPSIMD ~ 2:1 (engine-side SBUF ports, so they
            # don't contend with the DMA ports). PE stays free for
            # transpose+MLP.
            G = 2
            NG = (FRAMES + G - 1) // G
            frames_pool = ctx.enter_context(tc.tile_pool(name="frames", bufs=6))
            s_dve = state.tile([P, FFT_LEN], F32, tag="s_dve")
            s_gp = state.tile([P, FFT_LEN], F32, tag="s_gp")
            first_dve = True
            first_gp = True
            for g in range(NG):
                f0 = g * G
                nf = min(G, FRAMES - f0)
                xg = frames_pool.tile([P, G * FFT_LEN], F32, tag="xg")
                eng = nc.sync if g % 2 == 0 else nc.scalar
                eng.dma_start(
                    out=xg[:, : nf * FFT_LEN], in_=x3[:, f0 : f0 + nf, :]
                )
                for j in range(nf):
                    f = f0 + j
                    sl = xg[:, j * FFT_LEN : (j + 1) * FFT_LEN]
                    if f % 3 == 1:
                        if first_gp:
                            nc.gpsimd.tensor_copy(s_gp, sl)
                            first_gp = False
                        else:
                            nc.gpsimd.tensor_add(s_gp, s_gp, sl)
                    else:
                        if first_dve:
                            nc.vector.tensor_copy(s_dve, sl)
                            first_dve = False
                        else:
                            nc.vector.tensor_add(s_dve, s_dve, sl)
            nc.vector.tensor_add(s_sb, s_dve, s_gp)
        elif sum_mode == "dvpe":
            # DVE + PE split by frames. GPSIMD is useless here (it shares an
            # exclusive SBUF port lock with DVE), but PE reads SBUF through
            # its own ports and accumulates into PSUM via identity-stationary
            # matmuls, so DVE (21 frames @ ~2.3us) and PE (10 frames @
            # ~5.7us) genuinely run in parallel and together outpace the
            # ~0.35 frames/us DMA delivery rate.
            G = 2
            NG = (FRAMES + G - 1) // G
            frames_pool = ctx.enter_context(tc.tile_pool(name="frames", bufs=9))
            s_ps = ctx.enter_context(
                tc.tile_pool(name="s_psum", bufs=1, space="PSUM")
            )
            s_psum = s_ps.tile([P, FFT_LEN], F32)
            s_dve = state.tile([P, FFT_LEN], F32, tag="s_dve")
            # Tail scheduling: DVE's in-place add chain is serial, so giving
            # it ALL the last frames exposes ~5 back-to-back 2.3us adds after
            # the stream ends. Interleave instead: PE (kept warm by its
            # mid-stream frames) takes 27/29, DVE keeps 26/28/30 — the two
            # chains drain the last three groups in parallel.
            pe_frames = [
                f for f in range(FRAMES)
                if (f % 3 == 1 and f <= 25) or f in (27, 29)
            ]
            first_dve = True
            n_pe_done = 0
            for g in range(NG):
                f0 = g * G
                nf = min(G, FRAMES - f0)
                xg = frames_pool.tile([P, G * FFT_LEN], F32, tag="xg")
                eng = nc.sync if g % 2 == 0 else nc.scalar
                eng.dma_start(
                    out=xg[:, : nf * FFT_LEN], in_=x3[:, f0 : f0 + nf, :]
                )
                for j in range(nf):
                    f = f0 + j
                    sl = xg[:, j * FFT_LEN : (j + 1) * FFT_LEN]
                    if f in pe_frames:
                        for c in range(FFT_LEN // 512):
                            nc.tensor.matmul(
                                s_psum[:, c * 512 : (c + 1) * 512],
                                lhsT=ident,
                                rhs=sl[:, c * 512 : (c + 1) * 512],
                                start=(n_pe_done == 0),
                                stop=(n_pe_done == len(pe_frames) - 1),
                            )
                        n_pe_done += 1
                    else:
                        if first_dve:
                            nc.vector.tensor_copy(s_dve, sl)
                            first_dve = False
                        elif f == FRAMES - 1:
                            # final add quartered so merge q0 (and the first
                            # transposes behind it) can start ~1.7us earlier
                            QA = FFT_LEN // 4
                            for q in range(4):
                                qs = slice(q * QA, (q + 1) * QA)
                                nc.vector.tensor_add(
                                    s_dve[:, qs], s_dve[:, qs], sl[:, qs]
                                )
                        else:
                            nc.vector.tensor_add(s_dve, s_dve, sl)
            # merge: DVE reads the PE partial out of PSUM; four quarter-width
            # ops so the first transposes start ~0.6us after the final add
            # and interleave with the remaining merges. Output dtype f32r =
            # the rounding producer the BIR verifier requires for the f32r
            # transposes.
            Q = FFT_LEN // 4
            for q in range(4):
                qs = slice(q * Q, (q + 1) * Q)
                nc.vector.tensor_add(s_sb[:, qs], s_dve[:, qs], s_psum[:, qs])
        elif sum_mode == "pe":
            frames_pool = ctx.enter_context(tc.tile_pool(name="frames", bufs=6))
            s_ps = ctx.enter_context(
                tc.tile_pool(name="s_psum", bufs=1, space="PSUM")
            )
            s_psum = s_ps.tile([P, FFT_LEN], F32)
            NMM = FFT_LEN // 512  # 4 matmuls of N=512 per frame
            for f in range(FRAMES):
                xf = frames_pool.tile([P, FFT_LEN], F32, tag="xf")
                nc.sync.dma_start(out=xf, in_=x3[:, f, :])
                for c in range(NMM):
                    ms = slice(c * 512, (c + 1) * 512)
                    nc.tensor.matmul(
                        s_psum[:, ms],
                        lhsT=ident,
                        rhs=xf[:, ms],
                        start=(f == 0),
                        stop=(f == FRAMES - 1),
                    )
            nc.vector.tensor_copy(s_sb, s_psum)
        else:
            raise ValueError(f"unknown sum_mode {sum_mode}")

        # ---- transpose s -> sT (feature on partitions, batch on free) ----
        sT_dt = mybir.dt.bfloat16 if sum_mode == "dvpe" else F32
        sT_sb = state.tile([P, FFT_LEN], sT_dt, tag="sT_sb")
        if sum_mode == "dma":
            pjoin = ctx.enter_context(
                tc.tile_pool(name="pjoin", bufs=NCHUNK, space="PSUM")
            )
            W = FFT_LEN // NCHUNK
            for c in range(NCHUNK):
                # pre-join: PE observes accum-DMA lane c with a single-wait
                # dummy before any real transpose consumes this chunk.
                pj = pjoin.tile([1, 1], F32, tag="pj")
                col = s_sb[:, c * W : c * W + 1]
                nc.tensor.matmul(pj, lhsT=col, rhs=col, start=True, stop=True)
                for k in range(c * W // P, (c + 1) * W // P):
                    ks = slice(k * P, (k + 1) * P)
                    tp = pwork.tile([P, P], F32, tag="pw")
                    nc.tensor.transpose(tp, s_sb[:, ks], ident)
                    nc.vector.tensor_copy(sT_sb[:, ks], tp)
        else:
            tp_ident = ident_r if sum_mode == "dvpe" else ident
            for k in range(KCH):
                ks = slice(k * P, (k + 1) * P)
                tp = pwork.tile([P, P], s_dtype, tag="pw", name=f"tp{k}")
                nc.tensor.transpose(tp, s_sb[:, ks], tp_ident)
                nc.vector.tensor_copy(sT_sb[:, ks], tp)

        # ---- layer 1: h1T[m*128+j, b] = relu(sum_n W1c[n, m*128+j] sT[n, b] + b1) ----
        h1_sb = state.tile([P, H1], F32, tag="h1_sb")
        for m in range(2):
            h1p = pwork.tile([P, P], F32, tag="pw")
            for k in range(KCH):
                nc.tensor.matmul(
                    h1p,
                    lhsT=w1c(k, m),
                    rhs=sT_sb[:, k * P : (k + 1) * P],
                    start=(k == 0),
                    stop=(k == KCH - 1),
                )
            nc.scalar.activation(
                h1_sb[:, m * P : (m + 1) * P],
                h1p,
                mybir.ActivationFunctionType.Relu,
                bias=wpk[:, B10 + m : B10 + m + 1],
                scale=1.0,
            )

        # ---- layer 2 ----
        h2_sb = state.tile([P, H2], F32, tag="h2_sb")
        for m in range(2):
            h2p = pwork.tile([P, P], F32, tag="pw")
            for k in range(2):
                nc.tensor.matmul(
                    h2p,
                    lhsT=w2t(k, m),
                    rhs=h1_sb[:, k * P : (k + 1) * P],
                    start=(k == 0),
                    stop=(k == 1),
                )
            nc.scalar.activation(
                h2_sb[:, m * P : (m + 1) * P],
                h2p,
                mybir.ActivationFunctionType.Relu,
                bias=wpk[:, B20 + m : B20 + m + 1],
                scale=1.0,
            )

        # ---- layer 3 + sigmoid ----
        op = pout.tile([1, P], F32, tag="pw_o")
        for k in range(2):
            nc.tensor.matmul(
                op,
                lhsT=wpk[:, W3T0 + k : W3T0 + k + 1],
                rhs=h2_sb[:, k * P : (k + 1) * P],
                start=(k == 0),
                stop=(k == 1),
            )
        o_sb = state.tile([1, BS], F32, tag="o_sb")
        nc.scalar.activation(
            o_sb,
            op,
            mybir.ActivationFunctionType.Sigmoid,
            bias=wpk[0:1, B30 : B30 + 1],
            scale=1.0,
        )
        nc.gpsimd.dma_start(out=out_h.ap(), in_=o_sb)

    nc.compile()
    return nc


_NC_CACHE: dict = {}


def _get_nc(sum_mode: str = SUM_MODE) -> bass.Bass:
    if sum_mode not in _NC_CACHE:
        _NC_CACHE[sum_mode] = build_nc(sum_mode)
    return _NC_CACHE[sum_mode]


_HOST_CACHE: dict = {}


def _host_weights(W1, b1, W2, b2, W3, b3):
    key = (W1.__array_interface__["data"][0], W1.shape)
    if key in _HOST_CACHE:
        return _HOST_CACHE[key]
    import ml_dtypes

    n = np.arange(FFT_LEN)
    ang = (2.0 * np.pi / FFT_LEN) * ((n[:, None] * n[None, :]) % FFT_LEN)
    C = np.cos(ang)  # float64 [2048, 2048]
    W1c = (C @ W1.astype(np.float64).T / FRAMES).astype(np.float32)  # [2048, 256]
    W2T = W2.astype(np.float32).T  # [256, 256]
    W3T = W3.astype(np.float32).T.reshape(H2)  # [256]

    wpk = np.zeros((P, NW), dtype=np.float32)
    wpk[:, IDENT0 : IDENT0 + P] = np.eye(P, dtype=np.float32)
    for k in range(2):
        wpk[:, W2T0 + k * H2 : W2T0 + (k + 1) * H2] = W2T[k * P : (k + 1) * P, :]
    for k in range(2):
        wpk[:, W3T0 + k] = W3T[k * P : (k + 1) * P]
    for m in range(2):
        wpk[:, B10 + m] = b1.astype(np.float32)[m * P : (m + 1) * P]
        wpk[:, B20 + m] = b2.astype(np.float32)[m * P : (m + 1) * P]
    wpk[:, B30] = np.float32(b3.reshape(-1)[0])

    w1cb = np.zeros((P, NW1B), dtype=ml_dtypes.bfloat16)
    for k in range(KCH):
        w1cb[:, k * H1 : (k + 1) * H1] = W1c[k * P : (k + 1) * P, :].astype(
            ml_dtypes.bfloat16
        )

    pack = {"wpk": wpk, "w1cb": w1cb}
    _HOST_CACHE[key] = pack
    return pack


def kernel(x, W1, b1, W2, b2, W3, b3, _trace=False, _sum_mode=None):
    sum_mode = _sum_mode or SUM_MODE
    x = np.asarray(x, dtype=np.float32)
    pack = _host_weights(
        np.asarray(W1), np.asarray(b1), np.asarray(W2),
        np.asarray(b2), np.asarray(W3), np.asarray(b3),
    )
    in_maps = [
        {"x": np.ascontiguousarray(x[c * BS : (c + 1) * BS]), **pack}
        for c in range(NCORES)
    ]
    nc = _get_nc(sum_mode)
    res = run_bass_kernel_spmd(
        nc, in_maps, core_ids=list(range(NCORES)), trace=_trace
    )
    out = np.concatenate([res.results[c]["out"][0] for c in range(NCORES)])
    out = out.reshape(B, 1).astype(np.float32)
    if _trace:
        return out, res
    return out
